# revision 6
# baseline (speedup 1.0000x reference)
"""Trainium2 Bass kernel: attention with post-softmax relative bias + LayerNorm.

Problem (B=8, S=2048, E=256, H=8, d=32):
    q/k/v = x @ W{q,k,v}.T  (per batch)
    attn  = softmax(q k^T / sqrt(E)) + rel_bias        (bias added AFTER softmax)
    out   = LayerNorm(attn @ v)

Sharding: 1 batch per NeuronCore (8 cores), weights/bias replicated.

Per-core device algorithm (S^T layout so no transposes of the attention
matrix are ever needed and the softmax denominator comes out of the PE):
  - QT, KT = Wq^T.T @ xT etc  -> [256, 2048] SBUF, head h at partitions 32h:32h+32.
  - V = x @ Wv.T -> 16 tiles [128, 256] (seq on partitions).
  - For each (head h, i-half, j-block jb):
      ST[j, i] = K_h Q_h^T chunk  (PSUM [128, 1024], fp32r matmuls, K=32)
      expPT    = exp(ST / sqrt(E))          (ACT, PSUM->SBUF, no max-sub:
                                             logits are ~N(0, 0.04), safe)
      3 concurrent accumulation streams into one PSUM tile [65, 512] per
      512-chunk c (col-group tiling):
        rows  0-31: (exp @ V)^T        lhsT=V_h[jb]      rhs=expPT
        rows 32-63: (B @ V)^T          lhsT=V_h[jb]      rhs=Wh[:, u0:u0+512]
        row     64: sum_j exp          lhsT=ones[128,1]  rhs=expPT
      where B[i,j] = c_h[i-j+2047] (Toeplitz) and Wh[r, u] = c_h[u+127-r] is a
      host-precomputed sliding-window table: B^T tile (jb) = Wh[:, 1920-128jb : ...].
  - Finalize: PE-transpose [65, 128] chunks -> [128, 65], then
      out[i, 32h+d] = PV^T[i,d] / s_i + BV^T[i,d].
  - LayerNorm over E via bn_stats/bn_aggr, write out.

All SBUF pools stay open for the whole kernel (no SBUF address reuse across
phases: reuse creates WAR wait fan-ins that exceed the per-instruction
sync-wait budget in walrus codegen). Only PSUM pools are phase-scoped.
"""

import numpy as np

import concourse.bacc as bacc
import concourse.bass as bass
import concourse.tile as tile
from concourse import mybir
from concourse.masks import make_identity

B, S, E, H = 8, 2048, 256, 8
D = E // H  # 32
SCALE = float(E) ** -0.5
LN_EPS = 1e-5

WTAB_W = 2 * S - 128  # 3968
F32 = mybir.dt.float32
F32R = mybir.dt.float32r
BF16 = mybir.dt.bfloat16

N_JB = S // 128            # 16 j-blocks
N_HALF = 2                 # i processed in halves of 1024
HALF_W = S // N_HALF       # 1024
N_C = HALF_W // 512        # 2 512-chunks per half
N_Q = 4                    # 128-chunks per 512-chunk


def build_bass():
    nc = bacc.Bacc("TRN2")

    xT = nc.dram_tensor("xT", [E, S], F32, kind="ExternalInput")
    wqT = nc.dram_tensor("wqT", [E, E], F32, kind="ExternalInput")
    wkT = nc.dram_tensor("wkT", [E, E], F32, kind="ExternalInput")
    wvT = nc.dram_tensor("wvT", [E, E], F32, kind="ExternalInput")
    wtab = nc.dram_tensor("wtab", [H, 128, WTAB_W], F32, kind="ExternalInput")
    gamma = nc.dram_tensor("gamma", [E], F32, kind="ExternalInput")
    beta = nc.dram_tensor("beta", [E], F32, kind="ExternalInput")
    out = nc.dram_tensor("out", [S, E], F32, kind="ExternalOutput")

    with tile.TileContext(nc) as tc:
        with (
            tc.tile_pool(name="persist", bufs=1) as persist,
            tc.tile_pool(name="projsb", bufs=1) as projsb,
            tc.tile_pool(name="whf", bufs=1) as whfp,
            tc.tile_pool(name="whr", bufs=2) as whp,
            tc.tile_pool(name="expp", bufs=3) as expp,
            tc.tile_pool(name="fin", bufs=4) as finp,
            tc.tile_pool(name="ln", bufs=4) as lnp,
        ):
            # Persistent SBUF tensors
            qt_sb = [persist.tile([128, S], BF16, tag=f"qt{t}", name=f"qt{t}") for t in range(2)]
            kt_sb = [persist.tile([128, S], BF16, tag=f"kt{t}", name=f"kt{t}") for t in range(2)]
            v_sb = [persist.tile([128, H, D + 1], BF16, tag=f"v{j}", name=f"v{j}") for j in range(N_JB)]
            out_sb = [persist.tile([128, E], F32, tag=f"o{t}", name=f"o{t}") for t in range(N_JB)]
            ident = persist.tile([128, 128], F32, tag="ident", name="ident")
            gamma_sb = persist.tile([128, E], F32, tag="gamma", name="gamma")
            beta_sb = persist.tile([128, E], F32, tag="beta", name="beta")
            eps_sb = persist.tile([128, 1], F32, tag="eps", name="eps")

            make_identity(nc, ident[:, :])
            nc.vector.memset(eps_sb[:, :], LN_EPS)
            nc.sync.dma_start(
                out=gamma_sb[:, :],
                in_=bass.AP(tensor=gamma, offset=0, ap=[[0, 128], [1, E]]),
            )
            nc.sync.dma_start(
                out=beta_sb[:, :],
                in_=bass.AP(tensor=beta, offset=0, ap=[[0, 128], [1, E]]),
            )

            # ---------------- Phase 1: projections (plain fp32 matmuls) -----
            xt_sb = [projsb.tile([128, S], F32, tag=f"xt{t}", name=f"xt{t}") for t in range(2)]
            w_sb = {}
            for name, hdl in (("q", wqT), ("k", wkT), ("v", wvT)):
                w_sb[name] = [
                    projsb.tile([128, E], F32, tag=f"w{name}{t}", name=f"w{name}{t}")
                    for t in range(2)
                ]
                for t in range(2):
                    nc.sync.dma_start(
                        out=w_sb[name][t][:, :], in_=hdl[t * 128 : (t + 1) * 128, :]
                    )
            for t in range(2):
                nc.sync.dma_start(
                    out=xt_sb[t][:, :], in_=xT[t * 128 : (t + 1) * 128, :]
                )

            xt_r = [projsb.tile([128, S], F32R, tag=f"xtr{t}", name=f"xtr{t}") for t in range(2)]
            w_r = {}
            for t in range(2):
                nc.vector.tensor_copy(xt_r[t][:, :], xt_sb[t][:, :])
            for name in ("q", "k", "v"):
                w_r[name] = [
                    projsb.tile([128, E], F32R, tag=f"wr{name}{t}", name=f"wr{name}{t}")
                    for t in range(2)
                ]
                for t in range(2):
                    nc.vector.tensor_copy(w_r[name][t][:, :], w_sb[name][t][:, :])

            with tc.tile_pool(name="projps", bufs=3, space="PSUM") as projps:
                # QT / KT: out[e', i] = sum_e W[e', e] x^T[e, i]
                for name, dst in (("q", qt_sb), ("k", kt_sb)):
                    for t in range(2):  # output partition tile (e' half)
                        for c in range(4):  # 512-wide i chunk
                            ps = projps.tile([128, 512], F32, tag="pj", name="pj")
                            for k in range(2):
                                nc.tensor.matmul(
                                    ps[:, :],
                                    lhsT=w_r[name][k][:, t * 128 : (t + 1) * 128],
                                    rhs=xt_r[k][:, c * 512 : (c + 1) * 512],
                                    start=(k == 0),
                                    stop=(k == 1),
                                )
                            nc.vector.tensor_copy(
                                dst[t][:, c * 512 : (c + 1) * 512], ps[:, :]
                            )

                # V: out[j, e'] = sum_e x[j, e] Wv[e', e] -> lhsT = xT slice
                for j in range(N_JB):
                    ps = projps.tile([128, E], F32, tag="pv", name="pv")
                    for k in range(2):
                        nc.tensor.matmul(
                            ps[:, :],
                            lhsT=xt_r[k][:, j * 128 : (j + 1) * 128],
                            rhs=w_r["v"][k][:, :],
                            start=(k == 0),
                            stop=(k == 1),
                        )
                    nc.vector.tensor_copy(
                        v_sb[j][:, :, 0:D],
                        ps[:, :].rearrange("p (h d) -> p h d", h=H),
                    )
                    nc.vector.memset(v_sb[j][:, :, D : D + 1], 1.0)

            # ---------------- Phase 2: attention ----------------
            with (
                tc.tile_pool(name="stps", bufs=2, space="PSUM") as stps,
                tc.tile_pool(name="ops", bufs=2, space="PSUM") as ops,
                tc.tile_pool(name="tps", bufs=2, space="PSUM") as tps,
            ):
                for h in range(H):
                    ht, hh = divmod(h, 4)
                    kt_h = kt_sb[ht]
                    qt_h = qt_sb[ht]
                    wh_f = whfp.tile([128, WTAB_W], F32, tag="whf", name="whf")
                    nc.sync.dma_start(out=wh_f[:, :], in_=wtab[h, :, :])
                    wh_sb = whp.tile([128, WTAB_W], BF16, tag="wh", name="wh")
                    nc.vector.tensor_copy(wh_sb[:, :], wh_f[:, :])

                    for half in range(N_HALF):
                        i0 = half * HALF_W
                        opv = [
                            ops.tile([96, 512], F32, tag="opv", name="opv")
                            for _ in range(N_C)
                        ]
                        for jb in range(N_JB):
                            st = stps.tile([128, HALF_W], F32, tag="st", name="st")
                            for cc in range(N_C):
                                nc.tensor.matmul(
                                    st[:, cc * 512 : (cc + 1) * 512],
                                    lhsT=kt_h[
                                        32 * hh : 32 * hh + 32,
                                        jb * 128 : (jb + 1) * 128,
                                    ],
                                    rhs=qt_h[
                                        32 * hh : 32 * hh + 32,
                                        i0 + cc * 512 : i0 + (cc + 1) * 512,
                                    ],
                                    start=True,
                                    stop=True,
                                    tile_position=(32 * hh, 0),
                                )
                            ept = expp.tile([128, HALF_W], BF16, tag="ept", name="ept")
                            nc.scalar.activation(
                                ept[:, :],
                                st[:, :],
                                mybir.ActivationFunctionType.Exp,
                                scale=SCALE,
                            )
                            vh_ext = v_sb[jb][:, h, :]      # [128, 33]: V | ones
                            vh = v_sb[jb][:, h, 0:D]        # [128, 32]
                            for c in range(N_C):
                                ec = ept[:, c * 512 : (c + 1) * 512]
                                u0 = 1920 - 128 * jb + i0 + c * 512
                                # (exp @ V)^T rows 0-31 + denominator row 32
                                nc.tensor.matmul(
                                    opv[c][0:33, :],
                                    lhsT=vh_ext,
                                    rhs=ec,
                                    start=(jb == 0),
                                    stop=(jb == N_JB - 1),
                                    tile_position=(0, 0),
                                    skip_group_check=True,
                                )
                                # (B @ V)^T rows 64-95 from the sliding bias table
                                nc.tensor.matmul(
                                    opv[c][64:96, :],
                                    lhsT=vh,
                                    rhs=wh_sb[:, u0 : u0 + 512],
                                    start=(jb == 0),
                                    stop=(jb == N_JB - 1),
                                    tile_position=(0, 64),
                                    skip_group_check=True,
                                )

                        # finalize this (h, half)
                        for c in range(N_C):
                            o_sb = finp.tile([96, 512], F32, tag="osb", name="osb")
                            nc.vector.tensor_copy(o_sb[:, :], opv[c][:, :])
                            for q in range(N_Q):
                                it = (i0 + c * 512 + q * 128) // 128
                                pt = tps.tile([128, 96], F32, tag="pt", name="pt")
                                nc.tensor.transpose(
                                    pt[:, :],
                                    o_sb[:, q * 128 : (q + 1) * 128],
                                    ident[0:96, 0:96],
                                )
                                pb = finp.tile([128, 96], F32, tag="pb", name="pb")
                                nc.vector.tensor_copy(pb[:, :], pt[:, :])
                                recip = finp.tile([128, 1], F32, tag="rc", name="rc")
                                nc.vector.reciprocal(recip[:, :], pb[:, 32:33])
                                tmp = finp.tile([128, 32], F32, tag="tmp", name="tmp")
                                nc.vector.tensor_scalar_mul(
                                    tmp[:, :], in0=pb[:, 0:32], scalar1=recip[:, :]
                                )
                                nc.vector.tensor_add(
                                    out_sb[it][:, 32 * h : 32 * h + 32],
                                    in0=tmp[:, :],
                                    in1=pb[:, 64:96],
                                )

            # ---------------- Phase 3: LayerNorm + store ----------------
            for t in range(N_JB):
                xt_ = out_sb[t]
                stats = lnp.tile([128, 6], F32, tag="st", name="st")
                nc.vector.bn_stats(stats[:, :], xt_[:, :])
                mv = lnp.tile([128, 2], F32, tag="mv", name="mv")
                nc.vector.bn_aggr(mv[:, :], stats[:, :])
                rstd = lnp.tile([128, 1], F32, tag="rs", name="rs")
                nc.scalar.activation(
                    rstd[:, :],
                    mv[:, 1:2],
                    mybir.ActivationFunctionType.Sqrt,
                    bias=eps_sb[:, :],
                )
                nc.vector.reciprocal(rstd[:, :], rstd[:, :])
                nc.vector.tensor_scalar(
                    xt_[:, :],
                    in0=xt_[:, :],
                    scalar1=mv[:, 0:1],
                    scalar2=rstd[:, :],
                    op0=mybir.AluOpType.subtract,
                    op1=mybir.AluOpType.mult,
                )
                nc.vector.tensor_mul(xt_[:, :], in0=xt_[:, :], in1=gamma_sb[:, :])
                nc.vector.tensor_add(xt_[:, :], in0=xt_[:, :], in1=beta_sb[:, :])
                nc.sync.dma_start(
                    out=out[t * 128 : (t + 1) * 128, :], in_=xt_[:, :]
                )

    nc.finalize()
    return nc


_NC_CACHE = None


def _get_nc():
    global _NC_CACHE
    if _NC_CACHE is None:
        _NC_CACHE = build_bass()
    return _NC_CACHE


def _host_inputs(x, Wq, Wk, Wv, bias_table):
    """Build per-core input maps (all host-side preprocessing lives here)."""
    x = np.asarray(x, dtype=np.float32)
    wqT = np.ascontiguousarray(np.asarray(Wq, np.float32).T)
    wkT = np.ascontiguousarray(np.asarray(Wk, np.float32).T)
    wvT = np.ascontiguousarray(np.asarray(Wv, np.float32).T)
    bt = np.asarray(bias_table, np.float32)  # [2S-1, H]

    # Wh[h, r, u] = c_h[u + 127 - r],  c_h = bias_table[:, h]
    wtab = np.empty((H, 128, WTAB_W), np.float32)
    for h in range(H):
        sw = np.lib.stride_tricks.sliding_window_view(bt[:, h], WTAB_W)  # [128, W]
        wtab[h] = sw[::-1]
    return wqT, wkT, wvT, wtab


def kernel(x, Wq, Wk, Wv, bias_table, ln_gamma, ln_beta):
    from concourse.bass_utils import run_bass_kernel_spmd

    wqT, wkT, wvT, wtab = _host_inputs(x, Wq, Wk, Wv, bias_table)
    gamma = np.ascontiguousarray(np.asarray(ln_gamma, np.float32))
    beta = np.ascontiguousarray(np.asarray(ln_beta, np.float32))
    x = np.asarray(x, np.float32)

    in_maps = []
    for b in range(B):
        in_maps.append(
            {
                "xT": np.ascontiguousarray(x[b].T),
                "wqT": wqT,
                "wkT": wkT,
                "wvT": wvT,
                "wtab": wtab,
                "gamma": gamma,
                "beta": beta,
            }
        )

    nc = _get_nc()
    res = run_bass_kernel_spmd(nc, in_maps, core_ids=list(range(B)))
    return np.stack([r_["out"] for r_ in res.results]).astype(np.float32)


# revision 33
# speedup vs baseline: 98.3307x; 98.3307x over previous
"""Trainium2 Bass kernel: attention with post-softmax relative bias + LayerNorm.

Problem (B=8, S=2048, E=256, H=8, d=32):
    q/k/v = x @ W{q,k,v}.T  (per batch)
    attn  = softmax(q k^T / sqrt(E)) + rel_bias        (bias added AFTER softmax)
    out   = LayerNorm(attn @ v)

Sharding: 1 batch per NeuronCore (8 cores), weights/bias replicated.

Per-core device algorithm (S^T layout so no transposes of the attention
matrix are ever needed and the softmax denominator comes out of the PE):
  - QT, KT = Wq^T.T @ xT etc  -> [256, 2048] SBUF, head h at partitions 32h:32h+32.
  - V = x @ Wv.T -> 16 tiles [128, 256] (seq on partitions).
  - For each (head h, i-half, j-block jb):
      ST[j, i] = K_h Q_h^T chunk  (PSUM [128, 1024], fp32r matmuls, K=32)
      expPT    = exp(ST / sqrt(E))          (ACT, PSUM->SBUF, no max-sub:
                                             logits are ~N(0, 0.04), safe)
      3 concurrent accumulation streams into one PSUM tile [65, 512] per
      512-chunk c (col-group tiling):
        rows  0-31: (exp @ V)^T        lhsT=V_h[jb]      rhs=expPT
        rows 32-63: (B @ V)^T          lhsT=V_h[jb]      rhs=Wh[:, u0:u0+512]
        row     64: sum_j exp          lhsT=ones[128,1]  rhs=expPT
      where B[i,j] = c_h[i-j+2047] (Toeplitz) and Wh[r, u] = c_h[u+127-r] is a
      host-precomputed sliding-window table: B^T tile (jb) = Wh[:, 1920-128jb : ...].
  - Finalize: PE-transpose [65, 128] chunks -> [128, 65], then
      out[i, 32h+d] = PV^T[i,d] / s_i + BV^T[i,d].
  - LayerNorm over E via bn_stats/bn_aggr, write out.

All SBUF pools stay open for the whole kernel (no SBUF address reuse across
phases: reuse creates WAR wait fan-ins that exceed the per-instruction
sync-wait budget in walrus codegen). Only PSUM pools are phase-scoped.
"""

import numpy as np

import concourse.bacc as bacc
import concourse.bass as bass
import concourse.dve_ops as dve_ops
import concourse.tile as tile
from concourse import mybir
from concourse.dve_spec import C0, C1, C2, Spec, Src0, sq
from concourse.masks import make_identity
from concourse.tile_rust import add_dep_helper

B, S, E, H = 8, 2048, 256, 8
D = E // H  # 32
SCALE = float(E) ** -0.5
LN_EPS = 1e-5

WTAB_W = 2 * S - 128  # 3968
F32 = mybir.dt.float32
F32R = mybir.dt.float32r
BF16 = mybir.dt.bfloat16

N_JB = S // 128            # 16 j-blocks

# exp(z/16) ~= q(z)^2 with q quadratic (relative-minimax fit over z in [-7, 7];
# ~9e-4 max rel err, below the bf16 quantization of the exp tiles). Lets the
# DVE run part of the softmax exponentials in parallel with the ACT engine.
EXPA = (1.00007064, 3.14350307e-02, 4.86796494e-04)
DVE_EXP_JB = frozenset((3, 7, 11, 15))


def _exp_sqq_ref(in0, in1, s0, s1, imm2):
    z = in0.astype(np.float32)
    q = (imm2 * z + s1) * z + s0
    return (q * q).astype(np.float32)


def _register_exp_op():
    import re

    name = "EXP_SQQ_ANT"
    for op in dve_ops.OPS:
        if op.name == name:
            return op
    spec = Spec(body=sq((Src0 * C2 + C1) * Src0 + C0), reference=_exp_sqq_ref)
    row = dve_ops._CUSTOM_DVE_ROW_BASE + len(dve_ops.OPS)
    assert row < 0x20
    shas = {}
    for ver in ("v3", "v4"):
        trial = dve_ops.DveOp(name, spec, subdim=False, uops_sha={})
        dve_ops._SUB_OPCODE_FOR_NAME[name] = row
        try:
            trial.compile(ver)
            shas[ver] = trial.uops_sha.get(ver)
        except ValueError as e:
            m = re.search(r"([0-9a-f]{16}) ", str(e))
            assert m, f"could not parse sha from: {e}"
            shas[ver] = m.group(1) or m.group(2)
    op = dve_ops.DveOp(name, spec, subdim=False, uops_sha=shas)
    dve_ops.OPS.append(op)
    dve_ops.CUSTOM_DVE_SPECS[name] = spec
    dve_ops._SUB_OPCODE_FOR_NAME[name] = row
    return op


EXP_OP = _register_exp_op()
N_HALF = 2                 # i processed in halves of 1024
HALF_W = S // N_HALF       # 1024
N_C = HALF_W // 512        # 2 512-chunks per half
N_Q = 4                    # 128-chunks per 512-chunk


def build_bass():
    nc = bacc.Bacc("TRN2")

    xT = nc.dram_tensor("xT", [E, S], F32, kind="ExternalInput")
    wqT = nc.dram_tensor("wqT", [E, E], F32, kind="ExternalInput")
    wkT = nc.dram_tensor("wkT", [E, E], F32, kind="ExternalInput")
    wvT = nc.dram_tensor("wvT", [E, E], F32, kind="ExternalInput")
    wtab = nc.dram_tensor("wtab", [H, 128, WTAB_W], BF16, kind="ExternalInput")
    gamma = nc.dram_tensor("gamma", [E], F32, kind="ExternalInput")
    beta = nc.dram_tensor("beta", [E], F32, kind="ExternalInput")
    out = nc.dram_tensor("out", [S, E], F32, kind="ExternalOutput")

    with tile.TileContext(nc) as tc:
        with (
            tc.tile_pool(name="persist", bufs=1) as persist,
            tc.tile_pool(name="projsb", bufs=1) as projsb,
            tc.tile_pool(name="whr", bufs=2) as whp,
            tc.tile_pool(name="expp", bufs=3) as expp,
            tc.tile_pool(name="fin", bufs=6) as finp,
            tc.tile_pool(name="ln", bufs=4) as lnp,
        ):
            # Persistent SBUF tensors
            qt_sb = [persist.tile([128, S], BF16, tag=f"qt{t}", name=f"qt{t}") for t in range(2)]
            kt_sb = [persist.tile([128, S], BF16, tag=f"kt{t}", name=f"kt{t}") for t in range(2)]
            qt_r = [persist.tile([128, S], BF16, tag=f"qtr{t}", name=f"qtr{t}") for t in range(2)]
            kt_r = [persist.tile([128, S], BF16, tag=f"ktr{t}", name=f"ktr{t}") for t in range(2)]
            v_sb = [persist.tile([128, H, D + 1], BF16, tag=f"v{j}", name=f"v{j}") for j in range(N_JB)]
            out_sb = [persist.tile([128, E], F32, tag=f"o{t}", name=f"o{t}") for t in range(N_JB)]
            ident = persist.tile([128, 128], F32, tag="ident", name="ident")
            gamma_sb = persist.tile([128, E], F32, tag="gamma", name="gamma")
            beta_sb = persist.tile([128, E], F32, tag="beta", name="beta")
            eps_sb = persist.tile([128, 1], F32, tag="eps", name="eps")

            make_identity(nc, ident[:, :])
            nc.vector.memset(eps_sb[:, :], LN_EPS)
            nc.sync.dma_start(
                out=gamma_sb[:, :],
                in_=bass.AP(tensor=gamma, offset=0, ap=[[0, 128], [1, E]]),
            )
            nc.sync.dma_start(
                out=beta_sb[:, :],
                in_=bass.AP(tensor=beta, offset=0, ap=[[0, 128], [1, E]]),
            )

            # ---------------- Phase 1: projections (plain fp32 matmuls) -----
            xt_sb = [projsb.tile([128, S], F32, tag=f"xt{t}", name=f"xt{t}") for t in range(2)]
            w_sb = {}
            for name in ("q", "k", "v"):
                w_sb[name] = [
                    projsb.tile([128, E], F32, tag=f"w{name}{t}", name=f"w{name}{t}")
                    for t in range(2)
                ]
            # first-needed data first: xT chunk 0, Wq, then the rest
            for t in range(2):
                nc.sync.dma_start(
                    out=xt_sb[t][:, 0:512], in_=xT[t * 128 : (t + 1) * 128, 0:512]
                )
            for name, hdl in (("q", wqT), ("k", wkT), ("v", wvT)):
                for t in range(2):
                    nc.sync.dma_start(
                        out=w_sb[name][t][:, :], in_=hdl[t * 128 : (t + 1) * 128, :]
                    )
            for c in range(1, 4):
                for t in range(2):
                    nc.sync.dma_start(
                        out=xt_sb[t][:, c * 512 : (c + 1) * 512],
                        in_=xT[t * 128 : (t + 1) * 128, c * 512 : (c + 1) * 512],
                    )

            xt_r = [projsb.tile([128, S], F32R, tag=f"xtr{t}", name=f"xtr{t}") for t in range(2)]
            w_r = {}
            for c in range(4):
                for t in range(2):
                    nc.vector.tensor_copy(
                        xt_r[t][:, c * 512 : (c + 1) * 512],
                        xt_sb[t][:, c * 512 : (c + 1) * 512],
                    )
            for name in ("q", "k", "v"):
                w_r[name] = [
                    projsb.tile([128, E], F32R, tag=f"wr{name}{t}", name=f"wr{name}{t}")
                    for t in range(2)
                ]
                for t in range(2):
                    nc.vector.tensor_copy(w_r[name][t][:, :], w_sb[name][t][:, :])

            with tc.tile_pool(name="projps", bufs=3, space="PSUM") as projps:
                # QT / KT: out[e', i] = sum_e W[e', e] x^T[e, i]
                for name, dst in (("q", qt_sb), ("k", kt_sb)):
                    for t in range(2):  # output partition tile (e' half)
                        for c in range(4):  # 512-wide i chunk
                            ps = projps.tile([128, 512], F32, tag="pj", name="pj")
                            for k in range(2):
                                nc.tensor.matmul(
                                    ps[:, :],
                                    lhsT=w_r[name][k][:, t * 128 : (t + 1) * 128],
                                    rhs=xt_r[k][:, c * 512 : (c + 1) * 512],
                                    start=(k == 0),
                                    stop=(k == 1),
                                )
                            nc.vector.tensor_copy(
                                dst[t][:, c * 512 : (c + 1) * 512], ps[:, :]
                            )

                # V: out[j, e'] = sum_e x[j, e] Wv[e', e] -> lhsT = xT slice
                for j in range(N_JB):
                    ps = projps.tile([128, E], F32, tag="pv", name="pv")
                    for k in range(2):
                        nc.tensor.matmul(
                            ps[:, :],
                            lhsT=xt_r[k][:, j * 128 : (j + 1) * 128],
                            rhs=w_r["v"][k][:, :],
                            start=(k == 0),
                            stop=(k == 1),
                        )
                    nc.vector.tensor_copy(
                        v_sb[j][:, :, 0:D],
                        ps[:, :].rearrange("p (h d) -> p h d", h=H),
                    )
                    nc.vector.memset(v_sb[j][:, :, D : D + 1], 1.0)

            # Shifted replicas: rep[32a:32a+32] = orig[32((a+1)%4) : ...], so a
            # head's replica slice sits in a different PE row group than its
            # primary slice -> alternating QK matmuls overlap on the array.
            for t in range(2):
                for a in range(4):
                    srcg = 32 * ((a + 1) % 4)
                    nc.vector.tensor_copy(
                        qt_r[t][32 * a : 32 * a + 32, :],
                        qt_sb[t][srcg : srcg + 32, :],
                    )
                    nc.vector.tensor_copy(
                        kt_r[t][32 * a : 32 * a + 32, :],
                        kt_sb[t][srcg : srcg + 32, :],
                    )

            # ---------------- Phase 2: attention ----------------
            with (
                tc.tile_pool(name="stps", bufs=4, space="PSUM") as stps,
                tc.tile_pool(name="ops", bufs=2, space="PSUM") as ops,
                tc.tile_pool(name="tps", bufs=2, space="PSUM") as tps,
            ):
                for h in range(H):
                    ht, hh = divmod(h, 4)
                    kt_h = kt_sb[ht]
                    qt_h = qt_sb[ht]
                    wh_sb = whp.tile([128, WTAB_W], BF16, tag="wh", name="wh")
                    nc.sync.dma_start(out=wh_sb[:, :], in_=wtab[h, :, :])

                    for half in range(N_HALF):
                        i0 = half * HALF_W
                        opv = [
                            ops.tile([96, 512], F32, tag="opv", name="opv")
                            for _ in range(N_C)
                        ]
                        for jb in range(N_JB):
                            vh_ext = v_sb[jb][:, h, :]      # [128, 33]: V | ones
                            vh = v_sb[jb][:, h, 0:D]        # [128, 32]
                            hr = (hh + 3) % 4
                            for c in range(N_C):
                                st = stps.tile([128, 512], F32, tag="st", name="st")
                                if (jb + c) % 2 == 0:
                                    kt_src, qt_src, rg = kt_h, qt_h, hh
                                else:
                                    kt_src, qt_src, rg = kt_r[ht], qt_r[ht], hr
                                nc.tensor.matmul(
                                    st[:, :],
                                    lhsT=kt_src[
                                        32 * rg : 32 * rg + 32,
                                        jb * 128 : (jb + 1) * 128,
                                    ],
                                    rhs=qt_src[
                                        32 * rg : 32 * rg + 32,
                                        i0 + c * 512 : i0 + (c + 1) * 512,
                                    ],
                                    start=True,
                                    stop=True,
                                    tile_position=(32 * rg, 0),
                                )
                                ec = expp.tile([128, 512], BF16, tag="ept", name="ept")
                                if jb in DVE_EXP_JB:
                                    nc.vector._custom_dve(
                                        EXP_OP,
                                        out=ec[:, :],
                                        in0=st[:, :],
                                        s0=EXPA[0],
                                        s1=EXPA[1],
                                        imm2=EXPA[2],
                                    )
                                else:
                                    last_exp = nc.scalar.activation(
                                        ec[:, :],
                                        st[:, :],
                                        mybir.ActivationFunctionType.Exp,
                                        scale=SCALE,
                                    )
                                u0 = 1920 - 128 * jb + i0 + c * 512
                                # (exp @ V)^T rows 0-31 + denominator row 32
                                nc.tensor.matmul(
                                    opv[c][0:33, :],
                                    lhsT=vh_ext,
                                    rhs=ec[:, :],
                                    start=(jb == 0),
                                    stop=(jb == N_JB - 1),
                                    tile_position=(0, 0),
                                    skip_group_check=True,
                                )
                                # (B @ V)^T rows 64-95 from the sliding bias table
                                nc.tensor.matmul(
                                    opv[c][64:96, :],
                                    lhsT=vh,
                                    rhs=wh_sb[:, u0 : u0 + 512],
                                    start=(jb == 0),
                                    stop=(jb == N_JB - 1),
                                    tile_position=(0, 64),
                                    skip_group_check=True,
                                )

                        # finalize this (h, half): rows 0-31 PV^T, 32 s,
                        # 33 <- 1/s (computed below), 64-95 BV^T. The
                        # transpose carries 1/s to a [128,1] column.
                        for c in range(N_C):
                            o_sb = finp.tile([96, 512], F32, tag="osb", name="osb")
                            nc.vector.tensor_copy(o_sb[:, :], opv[c][:, :])
                            for q in range(N_Q):
                                it = (i0 + c * 512 + q * 128) // 128
                                pt = tps.tile([128, 96], F32, tag="pt", name="pt")
                                nc.tensor.transpose(
                                    pt[:, :],
                                    o_sb[:, q * 128 : (q + 1) * 128],
                                    ident[0:96, 0:96],
                                )
                                dst = out_sb[it][:, 32 * h : 32 * h + 32]
                                rc = finp.tile([128, 1], F32, tag="rc", name="rc")
                                nc.vector.reciprocal_approx_fast(
                                    rc[:, :], pt[:, 32:33]
                                )
                                nc.vector.tensor_copy(dst, pt[:, 64:96])
                                nc.vector.affine_then_add(
                                    dst,
                                    in0=pt[:, 0:32],
                                    in1=dst,
                                    scale=rc[:, :],
                                    bias=0.0,
                                )

            # ---------------- Phase 3: LayerNorm + store ----------------
            for t in range(N_JB):
                xt_ = out_sb[t]
                stats = lnp.tile([128, 6], F32, tag="st", name="st")
                nc.vector.bn_stats(stats[:, :], xt_[:, :])
                mv = lnp.tile([128, 2], F32, tag="mv", name="mv")
                nc.vector.bn_aggr(mv[:, :], stats[:, :])
                rstd = lnp.tile([128, 1], F32, tag="rs", name="rs")
                sqrt_i = nc.scalar.activation(
                    rstd[:, :],
                    mv[:, 1:2],
                    mybir.ActivationFunctionType.Sqrt,
                    bias=eps_sb[:, :],
                )
                # keep all Sqrt calls after the last Exp: one ACT table switch
                # instead of thrashing exp/sqrt sets at the tail
                add_dep_helper(sqrt_i.ins, last_exp.ins, sync=False)
                nc.vector.reciprocal(rstd[:, :], rstd[:, :])
                nc.vector.tensor_scalar(
                    xt_[:, :],
                    in0=xt_[:, :],
                    scalar1=mv[:, 0:1],
                    scalar2=rstd[:, :],
                    op0=mybir.AluOpType.subtract,
                    op1=mybir.AluOpType.mult,
                )
                nc.vector.tensor_mul(xt_[:, :], in0=xt_[:, :], in1=gamma_sb[:, :])
                nc.vector.tensor_add(xt_[:, :], in0=xt_[:, :], in1=beta_sb[:, :])
                nc.sync.dma_start(
                    out=out[t * 128 : (t + 1) * 128, :], in_=xt_[:, :]
                )

    nc.finalize()
    return nc


_NC_CACHE = None


def _get_nc():
    global _NC_CACHE
    if _NC_CACHE is None:
        _NC_CACHE = build_bass()
    return _NC_CACHE


def _host_inputs(x, Wq, Wk, Wv, bias_table):
    """Build per-core input maps (all host-side preprocessing lives here)."""
    x = np.asarray(x, dtype=np.float32)
    wqT = np.ascontiguousarray(np.asarray(Wq, np.float32).T)
    wkT = np.ascontiguousarray(np.asarray(Wk, np.float32).T)
    wvT = np.ascontiguousarray(np.asarray(Wv, np.float32).T)
    bt = np.asarray(bias_table, np.float32)  # [2S-1, H]

    # Wh[h, r, u] = c_h[u + 127 - r],  c_h = bias_table[:, h]
    import ml_dtypes

    wtab = np.empty((H, 128, WTAB_W), np.float32)
    for h in range(H):
        sw = np.lib.stride_tricks.sliding_window_view(bt[:, h], WTAB_W)  # [128, W]
        wtab[h] = sw[::-1]
    return wqT, wkT, wvT, wtab.astype(ml_dtypes.bfloat16)


def kernel(x, Wq, Wk, Wv, bias_table, ln_gamma, ln_beta):
    from concourse.bass_utils import run_bass_kernel_spmd

    wqT, wkT, wvT, wtab = _host_inputs(x, Wq, Wk, Wv, bias_table)
    gamma = np.ascontiguousarray(np.asarray(ln_gamma, np.float32))
    beta = np.ascontiguousarray(np.asarray(ln_beta, np.float32))
    x = np.asarray(x, np.float32)

    in_maps = []
    for b in range(B):
        in_maps.append(
            {
                "xT": np.ascontiguousarray(x[b].T),
                "wqT": wqT,
                "wkT": wkT,
                "wvT": wvT,
                "wtab": wtab,
                "gamma": gamma,
                "beta": beta,
            }
        )

    nc = _get_nc()
    res = run_bass_kernel_spmd(nc, in_maps, core_ids=list(range(B)))
    return np.stack([r_["out"] for r_ in res.results]).astype(np.float32)
